# revision 14
# baseline (speedup 1.0000x reference)
"""Trainium2 Bass kernel for single-head causal attention with QKV projections.

Reference computation (per batch element b):
    Q = Xq @ Wq; K = Xk @ Wk; V = Xv @ Wv          # [S, D] @ [D, H] -> [S, H]
    scores = Q @ K.T  (causal masked, strictly-upper -inf)
    probs  = softmax(scores / sqrt(S))
    out    = probs @ V                              # [S, H]

Sharding: batch-parallel across 8 NeuronCores (B == 8, one batch element per
core); weight matrices replicated.

Device algorithm per core (S=2048, D=1024, H=64, f32 throughout):
  1. Load X tiles [128, D] naturally; PE-transpose 128x128 blocks to build
     XT chunks [128(d), 512(s)]; project with W chunks to accumulate
     QT/KT/VT = W.T @ X.T as [64(h), 512(s)] PSUM tiles (contract d).
     Q and K projections are column-packed (tile_position cols 0/64) so the
     two matmuls run concurrently and share a PSUM bank.
  2. scoresT[k-block, q-block] = (KT chunk).T @ QT chunk -> PSUM [128, 512];
     row-packed in pairs (tile_position rows 0/64, K=64 contraction) using
     partition-duplicated copies of qT/kT; exp(scale*x) on ACT into SBUF;
     diagonal tiles multiplied by a 0/1 causal mask (exact zeros). No
     max-subtraction: |scores*scale| <= ~3 for gaussian inputs.
  3. V natural [128(k), 64] obtained by PE-transposing VT chunks; a ones
     column is appended (V' [128, 65]).
  4. outT[qc] += (V'[kc]).T @ expT(kc, qc)  -> PSUM [65, 512]; row 64
     accumulates the softmax denominator.
  5. PE-transpose outT blocks to [128, 65]; divide by the denominator
     column (DVE reciprocal + per-partition broadcast mul); DMA out.
"""

import os
import sys

import numpy as np

for _p in ("/opt/trn_rl_repo", os.path.expanduser("~/.axon_site/_ro/trn_rl_repo")):
    if os.path.isdir(_p) and _p not in sys.path:
        sys.path.insert(0, _p)

import concourse.bacc as bacc
import concourse.bass as bass
import concourse.mybir as mybir
import concourse.tile as tile
from concourse.bass_utils import run_bass_kernel_spmd
from concourse.masks import make_identity

B, S, D, H = 8, 2048, 1024, 64
N_CORES = 8
FP = mybir.dt.float32
SCALE = float(1.0 / np.sqrt(np.float32(S), dtype=np.float32))

SB = 4            # s-blocks of 512 over S
SBW = S // SB     # 512: s-block width
NKC = S // 128    # 16 k-chunks of 128
NE = D // 128     # 8 d-chunks of 128

PACK_PROJ = True     # column-pack Q|K projection matmuls
PACK_SCORES = True   # row-pack score matmul pairs (K=64)


def build_attention_nc(n_iters: int = 1) -> bass.Bass:
    nc = bacc.Bacc(
        "TRN2",
        target_bir_lowering=False,
        debug=False,
        enable_asserts=True,
        num_devices=N_CORES,
    )

    xq = nc.declare_dram_parameter("xq", [S, D], FP, isOutput=False)
    xk = nc.declare_dram_parameter("xk", [S, D], FP, isOutput=False)
    xv = nc.declare_dram_parameter("xv", [S, D], FP, isOutput=False)
    wq = nc.declare_dram_parameter("wq", [D, H], FP, isOutput=False)
    wk = nc.declare_dram_parameter("wk", [D, H], FP, isOutput=False)
    wv = nc.declare_dram_parameter("wv", [D, H], FP, isOutput=False)
    out = nc.declare_dram_parameter("out", [S, H], FP, isOutput=True)

    xs = {"q": xq, "k": xk, "v": xv}
    ws = {"q": wq, "k": wk, "v": wv}

    with tile.TileContext(nc) as tc:
        with tc.sbuf_pool(name="const_pool", bufs=1) as const_pool:
            ident = const_pool.tile([128, 128], FP, name="ident")
            make_identity(nc, ident)

            # Diagonal causal masks: mask_r[p, f] = 1.0 iff f >= 128*r + p.
            cmasks = []
            for r in range(4):
                cm = const_pool.tile([128, SBW], FP, name=f"cmask{r}")
                nc.gpsimd.memset(cm, 1.0)
                nc.gpsimd.affine_select(
                    out=cm,
                    in_=cm,
                    compare_op=mybir.AluOpType.is_ge,
                    fill=0.0,
                    base=-128 * r,
                    channel_multiplier=-1,
                    pattern=[[1, SBW]],
                )
                cmasks.append(cm)

            # Weights as [p(128), e(8), h(64)] so w_sb[:, e, :] is the [128, 64]
            # stationary for d-chunk e.
            w_sb = {}
            for t in ("q", "k", "v"):
                wt = const_pool.tile([128, NE, H], FP, name=f"w_{t}_sb")
                nc.sync.dma_start(
                    out=wt, in_=ws[t].rearrange("(e p) h -> p e h", p=128)
                )
                w_sb[t] = wt

            for it in range(n_iters):
                _emit_body(nc, tc, xs, out, w_sb, ident, cmasks, it)

    nc.compile()
    return nc


def _emit_body(nc, tc, xs, out, w_sb, ident, cmasks, it):
    sx = f"i{it}_"
    with (
        tc.sbuf_pool(name=f"{sx}persist", bufs=1) as persist,
        tc.sbuf_pool(name=f"{sx}exp_pool", bufs=1) as exp_pool,
    ):
        # qkT[sb]: partitions 0-63 hold qT, 64-127 hold kT (packed evac).
        # pair2[sb]: partitions 0-63 hold kT copy, 64-127 hold qT copy
        # (built by SBUF->SBUF DMA; enables row-packed score pairs).
        qkT = [persist.tile([128, SBW], FP, name=f"{sx}qkT{i}") for i in range(SB)]
        vT = [persist.tile([H, SBW], FP, name=f"{sx}vT{i}") for i in range(SB)]
        pair2 = [
            persist.tile([128, SBW], FP, name=f"{sx}pair2_{i}") for i in range(SB)
        ]
        # V natural with ones column, per k-chunk.
        vp = [persist.tile([128, H + 1], FP, name=f"{sx}vp{i}") for i in range(NKC)]
        for i in range(NKC):
            nc.gpsimd.memset(vp[i][:, H : H + 1], 1.0)
        # exp(scoresT) tiles keyed (kc, qc), causal only.
        et = {}
        for kc in range(NKC):
            for qc in range(kc // 4, SB):
                et[(kc, qc)] = exp_pool.tile([128, SBW], FP, name=f"{sx}et_{kc}_{qc}")

        def qT_lo(sb):  # qT at partitions 0-63
            return qkT[sb][0:H, :]

        def kT_hi(sb):  # kT at partitions 64-127
            return qkT[sb][H : 2 * H, :]

        # ---- Phase 1: load, transpose, project; Phase 2: scores + exp ----
        with (
            tc.sbuf_pool(name=f"{sx}xnat_pool", bufs=8) as xnat_pool,
            tc.sbuf_pool(name=f"{sx}xT_pool", bufs=4) as xT_pool,
            tc.psum_pool(name=f"{sx}tstage_pool", bufs=2) as tstage_pool,
            tc.psum_pool(name=f"{sx}proj_pool", bufs=2) as proj_pool,
            tc.psum_pool(name=f"{sx}sc_pool", bufs=2) as sc_pool,
        ):
            for sb in range(SB):
                xnat = {}
                for t in ("q", "k", "v"):
                    for c4 in range(4):
                        c = 4 * sb + c4
                        xt = xnat_pool.tile(
                            [128, D], FP, name=f"{sx}xnat_{t}{c}", tag="xnat"
                        )
                        nc.sync.dma_start(
                            out=xt, in_=xs[t][c * 128 : (c + 1) * 128, :]
                        )
                        xnat[(t, c4)] = xt

                def make_xT(t, e):
                    tstage = tstage_pool.tile(
                        [128, SBW], FP, name=f"{sx}tst_{t}{sb}{e}", tag="tst"
                    )
                    for c4 in range(4):
                        nc.tensor.transpose(
                            tstage[:, c4 * 128 : (c4 + 1) * 128],
                            xnat[(t, c4)][:, e * 128 : (e + 1) * 128],
                            ident,
                        )
                    xT = xT_pool.tile(
                        [128, SBW], FP, name=f"{sx}xT_{t}{sb}{e}", tag="xT"
                    )
                    if e % 2 == 0:
                        nc.vector.tensor_copy(xT, tstage)
                    else:
                        nc.scalar.copy(xT, tstage)
                    return xT

                if PACK_PROJ:
                    # Q into bank partitions 0-63 (col group 0), K into 64-127
                    # (col group 1): the two matmuls run concurrently.
                    pacc_qk = proj_pool.tile(
                        [128, SBW], FP, name=f"{sx}pacc_qk{sb}", tag="pacc_qk", bufs=2
                    )
                    pacc_v = proj_pool.tile(
                        [H, SBW], FP, name=f"{sx}pacc_v{sb}", tag="pacc_v", bufs=2
                    )
                    for e in range(NE):
                        xTq = make_xT("q", e)
                        xTk = make_xT("k", e)
                        nc.tensor.matmul(
                            pacc_qk[0:H, :], lhsT=w_sb["q"][:, e, :], rhs=xTq,
                            start=(e == 0), stop=(e == NE - 1),
                        )
                        nc.tensor.matmul(
                            pacc_qk[H : 2 * H, :], lhsT=w_sb["k"][:, e, :], rhs=xTk,
                            start=(e == 0), stop=(e == NE - 1),
                        )
                    for e in range(NE):
                        xTv = make_xT("v", e)
                        nc.tensor.matmul(
                            pacc_v, lhsT=w_sb["v"][:, e, :], rhs=xTv,
                            start=(e == 0), stop=(e == NE - 1),
                        )
                    nc.scalar.copy(qkT[sb], pacc_qk)
                    nc.scalar.copy(vT[sb], pacc_v)
                else:
                    accs = {}
                    for t in ("q", "k", "v"):
                        acc = proj_pool.tile(
                            [H, SBW], FP, name=f"{sx}pacc_{t}{sb}", tag="pacc"
                        )
                        for e in range(NE):
                            xT = make_xT(t, e)
                            nc.tensor.matmul(
                                acc, lhsT=w_sb[t][:, e, :], rhs=xT,
                                start=(e == 0), stop=(e == NE - 1),
                            )
                        accs[t] = acc
                    nc.scalar.copy(qkT[sb][0:H, :], accs["q"])
                    nc.scalar.copy(qkT[sb][H : 2 * H, :], accs["k"])
                    nc.scalar.copy(vT[sb], accs["v"])

                # kT copy to partitions 0-63, qT copy to 64-127 (via DMA,
                # which can move data across partitions).
                nc.sync.dma_start(out=pair2[sb][0:H, :], in_=kT_hi(sb))
                nc.sync.dma_start(out=pair2[sb][H : 2 * H, :], in_=qT_lo(sb))

                # Scores for qc == sb against every loaded k-chunk.
                qc = sb
                for kc in range(4 * sb + 4):
                    sc = sc_pool.tile(
                        [128, SBW], FP, name=f"{sx}sc_{kc}_{qc}", tag="sc"
                    )
                    ks = slice((kc % 4) * 128, (kc % 4 + 1) * 128)
                    if PACK_SCORES and kc % 2 == 1:
                        # row group 1: operands live on partitions 64-127
                        nc.tensor.matmul(
                            sc,
                            lhsT=qkT[kc // 4][H : 2 * H, ks],
                            rhs=pair2[qc][H : 2 * H, :],
                            start=True, stop=True,
                        )
                    else:
                        # row group 0: operands live on partitions 0-63
                        nc.tensor.matmul(
                            sc,
                            lhsT=pair2[kc // 4][0:H, ks],
                            rhs=qkT[qc][0:H, :],
                            start=True, stop=True,
                        )
                    e_tile = et[(kc, qc)]
                    nc.scalar.activation(
                        e_tile, sc, mybir.ActivationFunctionType.Exp, scale=SCALE
                    )
                    if kc // 4 == qc:
                        nc.vector.tensor_mul(e_tile, e_tile, cmasks[kc % 4])

        # ---- Phase 3: V natural (+ones) via PE transpose; PV accumulate ----
        with (
            tc.psum_pool(name=f"{sx}vnat_pool", bufs=2) as vnat_pool,
            tc.psum_pool(name=f"{sx}outT_pool", bufs=4) as outT_pool,
            tc.psum_pool(name=f"{sx}ot_pool", bufs=2) as ot_pool,
            tc.sbuf_pool(name=f"{sx}out_stage", bufs=4) as out_stage,
            tc.sbuf_pool(name=f"{sx}small_pool", bufs=4) as small_pool,
        ):
            for kc in range(NKC):
                vn = vnat_pool.tile([128, H], FP, name=f"{sx}vn{kc}", tag="vn")
                nc.tensor.transpose(
                    vn,
                    vT[kc // 4][:, (kc % 4) * 128 : (kc % 4 + 1) * 128],
                    ident[:H, :H],
                )
                nc.vector.tensor_copy(vp[kc][:, 0:H], vn)

            for qc in range(SB):
                oT = outT_pool.tile([H + 1, SBW], FP, name=f"{sx}oT{qc}", tag="oT")
                last_kc = 4 * qc + 3
                for kc in range(last_kc + 1):
                    nc.tensor.matmul(
                        oT,
                        lhsT=vp[kc],
                        rhs=et[(kc, qc)],
                        start=(kc == 0),
                        stop=(kc == last_kc),
                    )
                oT_sb = out_stage.tile(
                    [H + 1, SBW], FP, name=f"{sx}oTsb{qc}", tag="oTsb"
                )
                nc.scalar.copy(oT_sb, oT)

                for q4 in range(4):
                    ot = ot_pool.tile(
                        [128, H + 1], FP, name=f"{sx}ot{qc}{q4}", tag="ot"
                    )
                    nc.tensor.transpose(
                        ot,
                        oT_sb[:, q4 * 128 : (q4 + 1) * 128],
                        ident[: H + 1, : H + 1],
                    )
                    recip = small_pool.tile(
                        [128, 1], FP, name=f"{sx}rc{qc}{q4}", tag="rc"
                    )
                    nc.vector.reciprocal(recip, ot[:, H : H + 1])
                    obuf = small_pool.tile(
                        [128, H], FP, name=f"{sx}ob{qc}{q4}", tag="ob"
                    )
                    nc.vector.tensor_scalar_mul(obuf, ot[:, 0:H], recip)
                    qb = 4 * qc + q4
                    nc.sync.dma_start(
                        out=out[qb * 128 : (qb + 1) * 128, :], in_=obuf
                    )


_NC_CACHE = None


def _get_nc():
    global _NC_CACHE
    if _NC_CACHE is None:
        _NC_CACHE = build_attention_nc()
    return _NC_CACHE


def kernel(
    inputs_for_keys: np.ndarray,
    inputs_for_values: np.ndarray,
    inputs_for_queries: np.ndarray,
    K_matrix: np.ndarray,
    V_matrix: np.ndarray,
    Q_matrix: np.ndarray,
    _trace: bool = False,
):
    nc = _get_nc()

    f32 = np.float32
    ik = np.ascontiguousarray(inputs_for_keys, dtype=f32)
    iv = np.ascontiguousarray(inputs_for_values, dtype=f32)
    iq = np.ascontiguousarray(inputs_for_queries, dtype=f32)
    wk = np.ascontiguousarray(K_matrix, dtype=f32)
    wv = np.ascontiguousarray(V_matrix, dtype=f32)
    wq = np.ascontiguousarray(Q_matrix, dtype=f32)

    in_maps = [
        {"xq": iq[c], "xk": ik[c], "xv": iv[c], "wq": wq, "wk": wk, "wv": wv}
        for c in range(N_CORES)
    ]
    res = run_bass_kernel_spmd(nc, in_maps, list(range(N_CORES)), trace=_trace)
    outs = np.stack([res.results[c]["out"] for c in range(N_CORES)], axis=0)
    if _trace:
        return outs, res
    return outs
